# revision 12
# baseline (speedup 1.0000x reference)
"""Multi-head attention (B=4, S=2048, D=1024, H=16) on 8 Trainium2 cores.

Sharding: data-parallel over batch (4) x tensor-parallel over heads (2).
Core c handles batch c//2 and heads (c%2)*8 .. +8.  Each core computes a
partial output (its heads' contribution through the O-projection); the host
sums the two partials per batch and adds the output bias.

All device matmuls are arranged so the contraction dim sits on SBUF
partitions; activations are pre-transposed on the host so no on-device
transposes are needed.
"""

import numpy as np
from contextlib import ExitStack

import concourse.bass as bass
import concourse.tile as tile
from concourse import bacc, mybir
from concourse.bass import ts
from concourse.bass_utils import run_bass_kernel_spmd

P = 128
S = 2048          # sequence length
D = 1024          # model dim
DOUT = 512        # per-core projection width (8 heads x 64)
DK = 64           # head dim
B = 4
N_CORES = 8
F32 = mybir.dt.float32
FP = mybir.ActivationFunctionType

NKC = D // P      # 8 contraction chunks over model dim
NM = DOUT // P    # 4 dout chunks (also head pairs)
NQ = S // 512     # 4 query chunks of 512
NK16 = S // P     # 16 key chunks of 128

_cached_nc = None


def _emit(ctx: ExitStack, tc: "tile.TileContext", io: dict):
    nc = tc.nc

    qt_r = io["qt"].ap().rearrange("(c p) s -> p c s", p=P)      # [128, 8, 2048]
    kt_r = io["kt"].ap().rearrange("(c p) s -> p c s", p=P)
    vt_r = io["vt"].ap().rearrange("(c p) s -> p c s", p=P)
    wqt_r = io["wqt"].ap().rearrange("(c p) m -> p c m", p=P)    # [128, 8, 512]
    wkt_r = io["wkt"].ap().rearrange("(c p) m -> p c m", p=P)
    wvt_r = io["wvt"].ap().rearrange("(c p) m -> p c m", p=P)
    wot_r = io["wot"].ap().rearrange("(c p) n -> p c n", p=P)    # [128, 4, 1024]
    bq_r = io["bq"].ap().rearrange("(c p) -> p c", p=P)          # [128, 4]
    bk_r = io["bk"].ap().rearrange("(c p) -> p c", p=P)
    bv_ap = io["bv"].ap()                                        # [512]
    out_r = io["out"].ap().rearrange("(sc p) n -> p sc n", p=P)  # [128, 16, 1024]

    persist = ctx.enter_context(tc.tile_pool(name="persist", bufs=1))
    weights = ctx.enter_context(tc.tile_pool(name="weights", bufs=2))
    streams = ctx.enter_context(tc.tile_pool(name="streams", bufs=2))
    etp = ctx.enter_context(tc.tile_pool(name="etp", bufs=3))
    recipp = ctx.enter_context(tc.tile_pool(name="recipp", bufs=1))
    outp = ctx.enter_context(tc.tile_pool(name="outp", bufs=2))

    ps_proj = ctx.enter_context(tc.tile_pool(name="ps_proj", bufs=2, space="PSUM"))
    ps_st = ctx.enter_context(tc.tile_pool(name="ps_st", bufs=3, space="PSUM"))
    ps_av = ctx.enter_context(tc.tile_pool(name="ps_av", bufs=2, space="PSUM"))
    ps_sr = ctx.enter_context(tc.tile_pool(name="ps_sr", bufs=1, space="PSUM"))

    # ---- constants / biases -------------------------------------------------
    bq_sb = persist.tile([P, NM], F32, tag="bq")
    nc.sync.dma_start(out=bq_sb, in_=bq_r)
    bk_sb = persist.tile([P, NM], F32, tag="bk")
    nc.sync.dma_start(out=bk_sb, in_=bk_r)
    # bv replicated across partitions (DMA partition-broadcast, stride 0)
    bv_rep = persist.tile([P, DOUT], F32, tag="bvrep")
    bv_bcast = bass.AP(
        tensor=bv_ap.tensor, offset=bv_ap.offset, ap=[[0, P]] + list(bv_ap.ap)
    )
    nc.gpsimd.dma_start(out=bv_rep, in_=bv_bcast)
    ones_sb = persist.tile([P, 64], F32, tag="ones")
    nc.vector.memset(ones_sb, 1.0)

    # ---- persistent activations --------------------------------------------
    # qT / kT: [dout, s] as 4 chunk-tiles of [128, 2048] (chunk = head pair)
    qT = [persist.tile([P, S], F32, tag=f"qT{m}", name=f"qT{m}") for m in range(NM)]
    kT = [persist.tile([P, S], F32, tag=f"kT{m}", name=f"kT{m}") for m in range(NM)]
    # v: [s, dout] as 16 tiles of [128, 512] (tile = 128 seq positions)
    v_sb = [persist.tile([P, DOUT], F32, tag=f"v{i}", name=f"v{i}") for i in range(NK16)]
    # attn_outT: [dout, s] as 4 chunk-tiles (rows 0-63 even head, 64-127 odd)
    aoT = [persist.tile([P, S], F32, tag=f"aoT{m}", name=f"aoT{m}") for m in range(NM)]

    # ---- projections q^T, k^T -----------------------------------------------
    def project_T(src_r, w_r, bias_sb, dst):
        w_sb = weights.tile([P, NKC, DOUT], F32, tag="w")
        nc.sync.dma_start(out=w_sb, in_=w_r)
        for si in range(8):                       # 256-wide s chunks
            xin = streams.tile([P, NKC, 256], F32, tag="xin")
            nc.sync.dma_start(out=xin, in_=src_r[:, :, ts(si, 256)])
            for m in range(NM):
                ps = ps_proj.tile([P, 512], F32, tag="psv", name="ps")[:, 0:256]
                for kc in range(NKC):
                    nc.tensor.matmul(
                        ps,
                        lhsT=w_sb[:, kc, ts(m, P)],
                        rhs=xin[:, kc, :],
                        start=(kc == 0),
                        stop=(kc == NKC - 1),
                    )
                nc.vector.tensor_add(
                    out=dst[m][:, ts(si, 256)],
                    in0=ps,
                    in1=bias_sb[:, m : m + 1].to_broadcast([P, 256]),
                )

    project_T(qt_r, wqt_r, bq_sb, qT)
    project_T(kt_r, wkt_r, bk_sb, kT)

    # ---- projection v (natural layout) --------------------------------------
    wv_sb = weights.tile([P, NKC, DOUT], F32, tag="w")
    nc.sync.dma_start(out=wv_sb, in_=wvt_r)
    for g in range(8):
        vin = streams.tile([P, NKC, 256], F32, tag="xin")
        nc.sync.dma_start(out=vin, in_=vt_r[:, :, ts(g, 256)])
        for j in range(2):
            si16 = g * 2 + j
            ps = ps_proj.tile([P, 512], F32, tag="psv", name="psv")
            for kc in range(NKC):
                nc.tensor.matmul(
                    ps,
                    lhsT=vin[:, kc, ts(j, P)],
                    rhs=wv_sb[:, kc, :],
                    start=(kc == 0),
                    stop=(kc == NKC - 1),
                )
            nc.vector.tensor_add(out=v_sb[si16], in0=ps, in1=bv_rep)

    # ---- attention (scores kept transposed: tiles are S^T[k, q]) ------------
    # head pair pc: even head = partitions/cols 0-63, odd head = 64-127
    for pc in range(NM):
        for qi in range(NQ):
            av = ps_av.tile([P, 512], F32, tag="av", name="av")
            sums = ps_sr.tile([P, 512], F32, tag="sr", name="sums")  # rows 0/32: denoms
            for ki in range(NK16):
                stA = ps_st.tile([P, 512], F32, tag="st", name="stA")
                stB = ps_st.tile([P, 512], F32, tag="st", name="stB")
                nc.tensor.matmul(
                    stA,
                    lhsT=kT[pc][0:64, ts(ki, P)],
                    rhs=qT[pc][0:64, ts(qi, 512)],
                    start=True,
                    stop=True,
                )
                nc.tensor.matmul(
                    stB,
                    lhsT=kT[pc][64:128, ts(ki, P)],
                    rhs=qT[pc][64:128, ts(qi, 512)],
                    start=True,
                    stop=True,
                )
                etA = etp.tile([P, 512], F32, tag="et")
                etB = etp.tile([P, 512], F32, tag="et")
                nc.scalar.activation(out=etA, in_=stA, func=FP.Exp, scale=0.125)
                nc.scalar.activation(out=etB, in_=stB, func=FP.Exp, scale=0.125)
                first = ki == 0
                last = ki == NK16 - 1
                nc.tensor.matmul(
                    av[0:64],
                    lhsT=v_sb[ki][:, pc * P : pc * P + 64],
                    rhs=etA,
                    start=first,
                    stop=last,
                    skip_group_check=True,
                )
                nc.tensor.matmul(
                    av[64:128],
                    lhsT=v_sb[ki][:, pc * P + 64 : pc * P + 128],
                    rhs=etB,
                    start=first,
                    stop=last,
                    skip_group_check=True,
                )
                nc.tensor.matmul(
                    sums[0:1],
                    lhsT=ones_sb[:, 0:1],
                    rhs=etA,
                    start=first,
                    stop=last,
                    skip_group_check=True,
                )
                nc.tensor.matmul(
                    sums[32:33],
                    lhsT=ones_sb[:, 32:33],
                    rhs=etB,
                    start=first,
                    stop=last,
                    skip_group_check=True,
                )
            recip = recipp.tile([P, 512], F32, tag="recip")
            nc.vector.reciprocal(out=recip[0:1, :], in_=sums[0:1, :])
            nc.vector.reciprocal(out=recip[32:33, :], in_=sums[32:33, :])
            rep = ps_sr.tile([P, 512], F32, tag="sr", name="rep")
            nc.tensor.matmul(
                rep[0:64], lhsT=ones_sb[0:1, 0:64], rhs=recip[0:1, :],
                start=True, stop=True, skip_group_check=True,
            )
            nc.tensor.matmul(
                rep[64:128], lhsT=ones_sb[32:33, 0:64], rhs=recip[32:33, :],
                start=True, stop=True, skip_group_check=True,
            )
            rep_sb = recipp.tile([P, 512], F32, tag="rep_sb", name="rep_sb")
            nc.vector.tensor_copy(out=rep_sb, in_=rep)
            nc.vector.tensor_mul(out=aoT[pc][:, ts(qi, 512)], in0=av, in1=rep_sb)

    # ---- O-projection (partial; host adds the other core's half + bias) -----
    wo_sb = weights.tile([P, NM, D], F32, tag="w")
    nc.sync.dma_start(out=wo_sb, in_=wot_r)
    for si16 in range(NK16):
        for n2 in range(2):
            ps = ps_proj.tile([P, 512], F32, tag="psv", name="psv")
            for c in range(NM):
                nc.tensor.matmul(
                    ps,
                    lhsT=aoT[c][:, ts(si16, P)],
                    rhs=wo_sb[:, c, ts(n2, 512)],
                    start=(c == 0),
                    stop=(c == NM - 1),
                )
            osb = outp.tile([P, 512], F32, tag="osb")
            nc.vector.tensor_copy(out=osb, in_=ps)
            nc.sync.dma_start(out=out_r[:, si16, ts(n2, 512)], in_=osb)


def _build():
    global _cached_nc
    if _cached_nc is not None:
        return _cached_nc
    nc = bacc.Bacc("TRN2", target_bir_lowering=False, debug=False)
    io = {
        "qt": nc.dram_tensor("qt", [D, S], F32, kind="ExternalInput"),
        "kt": nc.dram_tensor("kt", [D, S], F32, kind="ExternalInput"),
        "vt": nc.dram_tensor("vt", [D, S], F32, kind="ExternalInput"),
        "wqt": nc.dram_tensor("wqt", [D, DOUT], F32, kind="ExternalInput"),
        "wkt": nc.dram_tensor("wkt", [D, DOUT], F32, kind="ExternalInput"),
        "wvt": nc.dram_tensor("wvt", [D, DOUT], F32, kind="ExternalInput"),
        "wot": nc.dram_tensor("wot", [DOUT, D], F32, kind="ExternalInput"),
        "bq": nc.dram_tensor("bq", [DOUT], F32, kind="ExternalInput"),
        "bk": nc.dram_tensor("bk", [DOUT], F32, kind="ExternalInput"),
        "bv": nc.dram_tensor("bv", [DOUT], F32, kind="ExternalInput"),
        "out": nc.dram_tensor("out", [S, D], F32, kind="ExternalOutput"),
    }
    with tile.TileContext(nc) as tc:
        with ExitStack() as ctx:
            _emit(ctx, tc, io)
    nc.compile()
    _cached_nc = nc
    return nc


def make_in_maps(Q, K, V, Wq, bq, Wk, bk, Wv, bv, Wo):
    f = lambda a: np.ascontiguousarray(a, dtype=np.float32)
    in_maps = []
    for c in range(N_CORES):
        b = c // 2
        lo = (c % 2) * DOUT
        sl = slice(lo, lo + DOUT)
        in_maps.append(
            {
                "qt": f(Q[b].T),
                "kt": f(K[b].T),
                "vt": f(V[b].T),
                "wqt": f(Wq[sl, :].T),
                "wkt": f(Wk[sl, :].T),
                "wvt": f(Wv[sl, :].T),
                "wot": f(Wo[:, sl].T),
                "bq": f(bq[sl]),
                "bk": f(bk[sl]),
                "bv": f(bv[sl]),
            }
        )
    return in_maps


def gather_output(results, bo):
    out = np.empty((B, S, D), dtype=np.float32)
    bo = np.asarray(bo, dtype=np.float32)
    for b in range(B):
        out[b] = results[2 * b]["out"] + results[2 * b + 1]["out"] + bo
    return out


def _numpy_fallback(Q, K, V, mask, Wq, bq, Wk, bk, Wv, bv, Wo, bo):
    """Exact reference math in numpy (only used if mask isn't all-ones)."""
    H, dk = 16, 64
    out = np.empty((B, S, D), dtype=np.float32)
    for b in range(B):
        q = (Q[b] @ Wq.T + bq).reshape(S, H, dk).transpose(1, 0, 2)
        k = (K[b] @ Wk.T + bk).reshape(S, H, dk).transpose(1, 0, 2)
        v = (V[b] @ Wv.T + bv).reshape(S, H, dk).transpose(1, 0, 2)
        o = np.empty((H, S, dk), dtype=np.float32)
        for h in range(H):
            s = (q[h] @ k[h].T) / np.sqrt(np.float32(dk))
            s = np.where(mask[b] == 0, np.float32(-1.0e9), s)
            s = s - s.max(axis=-1, keepdims=True)
            e = np.exp(s)
            a = e / e.sum(axis=-1, keepdims=True)
            o[h] = a @ v[h]
        out[b] = o.transpose(1, 0, 2).reshape(S, H * dk) @ Wo.T + bo
    return out


def kernel(Q, K, V, mask, Wq, bq, Wk, bk, Wv, bv, Wo, bo):
    Q = np.asarray(Q, dtype=np.float32)
    K = np.asarray(K, dtype=np.float32)
    V = np.asarray(V, dtype=np.float32)
    Wq = np.asarray(Wq, dtype=np.float32)
    Wk = np.asarray(Wk, dtype=np.float32)
    Wv = np.asarray(Wv, dtype=np.float32)
    Wo = np.asarray(Wo, dtype=np.float32)
    bq = np.asarray(bq, dtype=np.float32)
    bk = np.asarray(bk, dtype=np.float32)
    bv = np.asarray(bv, dtype=np.float32)
    bo = np.asarray(bo, dtype=np.float32)
    mask_np = np.asarray(mask)

    if not np.all(mask_np != 0):
        return _numpy_fallback(Q, K, V, mask_np, Wq, bq, Wk, bk, Wv, bv, Wo, bo)

    nc = _build()
    in_maps = make_in_maps(Q, K, V, Wq, bq, Wk, bk, Wv, bv, Wo)
    res = run_bass_kernel_spmd(nc, in_maps, list(range(N_CORES))).results
    return gather_output(res, bo)


# revision 13
# speedup vs baseline: 1.8124x; 1.8124x over previous
"""Multi-head attention (B=4, S=2048, D=1024, H=16) on 8 Trainium2 cores.

Sharding: data-parallel over batch (4) x tensor-parallel over heads (2).
Core c handles batch c//2 and heads (c%2)*8 .. +8.  Each core computes a
partial output (its heads' contribution through the O-projection); the host
sums the two partials per batch and adds the output bias.

All device matmuls are arranged so the contraction dim sits on SBUF
partitions; activations are pre-transposed on the host so no on-device
transposes are needed.  Matmul operands are bf16 (PSUM accumulation stays
f32, as does the softmax normalization), which quadruples TensorE
throughput vs fp32's two-pass mode.
"""

import numpy as np
from contextlib import ExitStack

import ml_dtypes
import concourse.bass as bass
import concourse.tile as tile
from concourse import bacc, mybir
from concourse.bass import ts
from concourse.bass_utils import run_bass_kernel_spmd

P = 128
S = 2048          # sequence length
D = 1024          # model dim
DOUT = 512        # per-core projection width (8 heads x 64)
DK = 64           # head dim
B = 4
N_CORES = 8
F32 = mybir.dt.float32
BF16 = mybir.dt.bfloat16
FP = mybir.ActivationFunctionType

NKC = D // P      # 8 contraction chunks over model dim
NM = DOUT // P    # 4 dout chunks (also head pairs)
NQ = S // 512     # 4 query chunks of 512
NK16 = S // P     # 16 key chunks of 128

_cached_nc = None


def _emit(ctx: ExitStack, tc: "tile.TileContext", io: dict):
    nc = tc.nc

    qt_r = io["qt"].ap().rearrange("(c p) s -> p c s", p=P)      # [128, 8, 2048]
    kt_r = io["kt"].ap().rearrange("(c p) s -> p c s", p=P)
    vt_r = io["vt"].ap().rearrange("(c p) s -> p c s", p=P)
    wqt_r = io["wqt"].ap().rearrange("(c p) m -> p c m", p=P)    # [128, 8, 512]
    wkt_r = io["wkt"].ap().rearrange("(c p) m -> p c m", p=P)
    wvt_r = io["wvt"].ap().rearrange("(c p) m -> p c m", p=P)
    wot_r = io["wot"].ap().rearrange("(c p) n -> p c n", p=P)    # [128, 4, 1024]
    bq_r = io["bq"].ap().rearrange("(c p) -> p c", p=P)          # [128, 4]
    bk_r = io["bk"].ap().rearrange("(c p) -> p c", p=P)
    bv_ap = io["bv"].ap()                                        # [512]
    out_r = io["out"].ap().rearrange("(sc p) n -> p sc n", p=P)  # [128, 16, 1024]

    persist = ctx.enter_context(tc.tile_pool(name="persist", bufs=1))
    weights = ctx.enter_context(tc.tile_pool(name="weights", bufs=2))
    streams = ctx.enter_context(tc.tile_pool(name="streams", bufs=3))
    etp = ctx.enter_context(tc.tile_pool(name="etp", bufs=4))
    recipp = ctx.enter_context(tc.tile_pool(name="recipp", bufs=2))
    outp = ctx.enter_context(tc.tile_pool(name="outp", bufs=3))
    dramp = ctx.enter_context(tc.tile_pool(name="dramp", bufs=2, space="DRAM"))

    ps_proj = ctx.enter_context(tc.tile_pool(name="ps_proj", bufs=2, space="PSUM"))
    ps_st = ctx.enter_context(tc.tile_pool(name="ps_st", bufs=2, space="PSUM"))
    ps_av = ctx.enter_context(tc.tile_pool(name="ps_av", bufs=2, space="PSUM"))
    ps_sums = ctx.enter_context(tc.tile_pool(name="ps_sums", bufs=2, space="PSUM"))

    # ---- constants / biases -------------------------------------------------
    bq_sb = persist.tile([P, NM], F32, tag="bq")
    nc.sync.dma_start(out=bq_sb, in_=bq_r)
    bk_sb = persist.tile([P, NM], F32, tag="bk")
    nc.sync.dma_start(out=bk_sb, in_=bk_r)
    # bv replicated across partitions (DMA partition-broadcast, stride 0)
    bv_rep = persist.tile([P, DOUT], F32, tag="bvrep")
    bv_bcast = bass.AP(
        tensor=bv_ap.tensor, offset=bv_ap.offset, ap=[[0, P]] + list(bv_ap.ap)
    )
    nc.gpsimd.dma_start(out=bv_rep, in_=bv_bcast)
    ones_sb = persist.tile([P, 64], BF16, tag="ones")
    nc.vector.memset(ones_sb, 1.0)

    # ---- persistent activations (bf16) --------------------------------------
    # qT / kT: [dout, s] as 4 chunk-tiles of [128, 2048] (chunk = head pair)
    qT = [persist.tile([P, S], BF16, tag=f"qT{m}", name=f"qT{m}") for m in range(NM)]
    kT = [persist.tile([P, S], BF16, tag=f"kT{m}", name=f"kT{m}") for m in range(NM)]
    # v: [s, dout] as 16 tiles of [128, 512] (tile = 128 seq positions)
    v_sb = [
        persist.tile([P, DOUT], BF16, tag=f"v{i}", name=f"v{i}") for i in range(NK16)
    ]
    # attn_outT: [dout, s] as 4 chunk-tiles (rows 0-63 even head, 64-127 odd)
    aoT = [persist.tile([P, S], BF16, tag=f"aoT{m}", name=f"aoT{m}") for m in range(NM)]

    # ---- projections q^T, k^T -----------------------------------------------
    def project_T(src_r, w_r, bias_sb, dst):
        w_sb = weights.tile([P, NKC, DOUT], BF16, tag="w")
        nc.sync.dma_start(out=w_sb, in_=w_r)
        for si in range(8):                       # 256-wide s chunks
            xin = streams.tile([P, NKC, 256], BF16, tag="xin")
            nc.sync.dma_start(out=xin, in_=src_r[:, :, ts(si, 256)])
            for m in range(NM):
                ps = ps_proj.tile([P, 512], F32, tag="psv", name="ps")[:, 0:256]
                for kc in range(NKC):
                    nc.tensor.matmul(
                        ps,
                        lhsT=w_sb[:, kc, ts(m, P)],
                        rhs=xin[:, kc, :],
                        start=(kc == 0),
                        stop=(kc == NKC - 1),
                    )
                nc.vector.tensor_add(
                    out=dst[m][:, ts(si, 256)],
                    in0=ps,
                    in1=bias_sb[:, m : m + 1].to_broadcast([P, 256]),
                )

    project_T(qt_r, wqt_r, bq_sb, qT)
    project_T(kt_r, wkt_r, bk_sb, kT)

    # ---- projection v (natural layout) --------------------------------------
    wv_sb = weights.tile([P, NKC, DOUT], BF16, tag="w")
    nc.sync.dma_start(out=wv_sb, in_=wvt_r)
    for g in range(8):
        vin = streams.tile([P, NKC, 256], BF16, tag="xin")
        nc.sync.dma_start(out=vin, in_=vt_r[:, :, ts(g, 256)])
        for j in range(2):
            si16 = g * 2 + j
            ps = ps_proj.tile([P, 512], F32, tag="psv", name="psv")
            for kc in range(NKC):
                nc.tensor.matmul(
                    ps,
                    lhsT=vin[:, kc, ts(j, P)],
                    rhs=wv_sb[:, kc, :],
                    start=(kc == 0),
                    stop=(kc == NKC - 1),
                )
            nc.vector.tensor_add(out=v_sb[si16], in0=ps, in1=bv_rep)

    # ---- attention (scores kept transposed: tiles are S^T[k, q]) ------------
    # head pair pc: even head = partitions/cols 0-63, odd head = 64-127
    for pc in range(NM):
        for qi in range(NQ):
            av = ps_av.tile([P, 512], F32, tag="av", name="av")
            sums = ps_sums.tile([P, 512], F32, tag="sums", name="sums")  # rows 0/32
            for ki in range(NK16):
                stA = ps_st.tile([P, 512], F32, tag="st", name="stA")
                stB = ps_st.tile([P, 512], F32, tag="st", name="stB")
                nc.tensor.matmul(
                    stA,
                    lhsT=kT[pc][0:64, ts(ki, P)],
                    rhs=qT[pc][0:64, ts(qi, 512)],
                    start=True,
                    stop=True,
                )
                nc.tensor.matmul(
                    stB,
                    lhsT=kT[pc][64:128, ts(ki, P)],
                    rhs=qT[pc][64:128, ts(qi, 512)],
                    start=True,
                    stop=True,
                )
                etA = etp.tile([P, 512], BF16, tag="et", name="etA")
                etB = etp.tile([P, 512], BF16, tag="et", name="etB")
                nc.scalar.activation(out=etA, in_=stA, func=FP.Exp, scale=0.125)
                nc.scalar.activation(out=etB, in_=stB, func=FP.Exp, scale=0.125)
                first = ki == 0
                last = ki == NK16 - 1
                nc.tensor.matmul(
                    av[0:64],
                    lhsT=v_sb[ki][:, pc * P : pc * P + 64],
                    rhs=etA,
                    start=first,
                    stop=last,
                    skip_group_check=True,
                )
                nc.tensor.matmul(
                    av[64:128],
                    lhsT=v_sb[ki][:, pc * P + 64 : pc * P + 128],
                    rhs=etB,
                    start=first,
                    stop=last,
                    skip_group_check=True,
                )
                nc.tensor.matmul(
                    sums[0:1],
                    lhsT=ones_sb[:, 0:1],
                    rhs=etA,
                    start=first,
                    stop=last,
                    skip_group_check=True,
                )
                nc.tensor.matmul(
                    sums[32:33],
                    lhsT=ones_sb[:, 32:33],
                    rhs=etB,
                    start=first,
                    stop=last,
                    skip_group_check=True,
                )
            # softmax denominators: 1/sums, replicated across partitions via a
            # DRAM bounce (DRAM sources allow stride-0 partition broadcast)
            recip = recipp.tile([P, 512], F32, tag="recip", name="recip")
            nc.vector.reciprocal(out=recip[0:1, :], in_=sums[0:1, :])
            nc.vector.reciprocal(out=recip[32:33, :], in_=sums[32:33, :])
            scr = dramp.tile([2, 512], F32, tag="scr", name="scr")
            nc.sync.dma_start(out=scr[0:1, :], in_=recip[0:1, :])
            nc.sync.dma_start(out=scr[1:2, :], in_=recip[32:33, :])
            rep_sb = recipp.tile([P, 512], F32, tag="rep_sb", name="rep_sb")
            s0 = scr[0:1, :]
            s1 = scr[1:2, :]
            nc.sync.dma_start(
                out=rep_sb[0:64, :],
                in_=bass.AP(
                    tensor=s0.tensor, offset=s0.offset, ap=[[0, 64]] + list(s0.ap[1:])
                ),
            )
            nc.sync.dma_start(
                out=rep_sb[64:128, :],
                in_=bass.AP(
                    tensor=s1.tensor, offset=s1.offset, ap=[[0, 64]] + list(s1.ap[1:])
                ),
            )
            nc.vector.tensor_mul(out=aoT[pc][:, ts(qi, 512)], in0=av, in1=rep_sb)

    # ---- O-projection (partial; host adds the other core's half + bias) -----
    wo_sb = weights.tile([P, NM, D], BF16, tag="w")
    nc.sync.dma_start(out=wo_sb, in_=wot_r)
    for si16 in range(NK16):
        for n2 in range(2):
            ps = ps_proj.tile([P, 512], F32, tag="psv", name="psv")
            for c in range(NM):
                nc.tensor.matmul(
                    ps,
                    lhsT=aoT[c][:, ts(si16, P)],
                    rhs=wo_sb[:, c, ts(n2, 512)],
                    start=(c == 0),
                    stop=(c == NM - 1),
                )
            osb = outp.tile([P, 512], F32, tag="osb")
            nc.vector.tensor_copy(out=osb, in_=ps)
            nc.sync.dma_start(out=out_r[:, si16, ts(n2, 512)], in_=osb)


def _build():
    global _cached_nc
    if _cached_nc is not None:
        return _cached_nc
    nc = bacc.Bacc("TRN2", target_bir_lowering=False, debug=False)
    io = {
        "qt": nc.dram_tensor("qt", [D, S], BF16, kind="ExternalInput"),
        "kt": nc.dram_tensor("kt", [D, S], BF16, kind="ExternalInput"),
        "vt": nc.dram_tensor("vt", [D, S], BF16, kind="ExternalInput"),
        "wqt": nc.dram_tensor("wqt", [D, DOUT], BF16, kind="ExternalInput"),
        "wkt": nc.dram_tensor("wkt", [D, DOUT], BF16, kind="ExternalInput"),
        "wvt": nc.dram_tensor("wvt", [D, DOUT], BF16, kind="ExternalInput"),
        "wot": nc.dram_tensor("wot", [DOUT, D], BF16, kind="ExternalInput"),
        "bq": nc.dram_tensor("bq", [DOUT], F32, kind="ExternalInput"),
        "bk": nc.dram_tensor("bk", [DOUT], F32, kind="ExternalInput"),
        "bv": nc.dram_tensor("bv", [DOUT], F32, kind="ExternalInput"),
        "out": nc.dram_tensor("out", [S, D], F32, kind="ExternalOutput"),
    }
    with tile.TileContext(nc) as tc:
        with ExitStack() as ctx:
            _emit(ctx, tc, io)
    nc.compile()
    _cached_nc = nc
    return nc


def make_in_maps(Q, K, V, Wq, bq, Wk, bk, Wv, bv, Wo):
    bf = lambda a: np.ascontiguousarray(np.asarray(a, np.float32)).astype(
        ml_dtypes.bfloat16
    )
    f = lambda a: np.ascontiguousarray(a, dtype=np.float32)
    in_maps = []
    for c in range(N_CORES):
        b = c // 2
        lo = (c % 2) * DOUT
        sl = slice(lo, lo + DOUT)
        in_maps.append(
            {
                "qt": bf(np.asarray(Q, np.float32)[b].T),
                "kt": bf(np.asarray(K, np.float32)[b].T),
                "vt": bf(np.asarray(V, np.float32)[b].T),
                "wqt": bf(np.asarray(Wq, np.float32)[sl, :].T),
                "wkt": bf(np.asarray(Wk, np.float32)[sl, :].T),
                "wvt": bf(np.asarray(Wv, np.float32)[sl, :].T),
                "wot": bf(np.asarray(Wo, np.float32)[:, sl].T),
                "bq": f(bq[sl]),
                "bk": f(bk[sl]),
                "bv": f(bv[sl]),
            }
        )
    return in_maps


def gather_output(results, bo):
    out = np.empty((B, S, D), dtype=np.float32)
    bo = np.asarray(bo, dtype=np.float32)
    for b in range(B):
        out[b] = results[2 * b]["out"] + results[2 * b + 1]["out"] + bo
    return out


def _numpy_fallback(Q, K, V, mask, Wq, bq, Wk, bk, Wv, bv, Wo, bo):
    """Exact reference math in numpy (only used if mask isn't all-ones)."""
    H, dk = 16, 64
    out = np.empty((B, S, D), dtype=np.float32)
    for b in range(B):
        q = (Q[b] @ Wq.T + bq).reshape(S, H, dk).transpose(1, 0, 2)
        k = (K[b] @ Wk.T + bk).reshape(S, H, dk).transpose(1, 0, 2)
        v = (V[b] @ Wv.T + bv).reshape(S, H, dk).transpose(1, 0, 2)
        o = np.empty((H, S, dk), dtype=np.float32)
        for h in range(H):
            s = (q[h] @ k[h].T) / np.sqrt(np.float32(dk))
            s = np.where(mask[b] == 0, np.float32(-1.0e9), s)
            s = s - s.max(axis=-1, keepdims=True)
            e = np.exp(s)
            a = e / e.sum(axis=-1, keepdims=True)
            o[h] = a @ v[h]
        out[b] = o.transpose(1, 0, 2).reshape(S, H * dk) @ Wo.T + bo
    return out


def kernel(Q, K, V, mask, Wq, bq, Wk, bk, Wv, bv, Wo, bo):
    Q = np.asarray(Q, dtype=np.float32)
    K = np.asarray(K, dtype=np.float32)
    V = np.asarray(V, dtype=np.float32)
    Wq = np.asarray(Wq, dtype=np.float32)
    Wk = np.asarray(Wk, dtype=np.float32)
    Wv = np.asarray(Wv, dtype=np.float32)
    Wo = np.asarray(Wo, dtype=np.float32)
    bq = np.asarray(bq, dtype=np.float32)
    bk = np.asarray(bk, dtype=np.float32)
    bv = np.asarray(bv, dtype=np.float32)
    bo = np.asarray(bo, dtype=np.float32)
    mask_np = np.asarray(mask)

    if not np.all(mask_np != 0):
        return _numpy_fallback(Q, K, V, mask_np, Wq, bq, Wk, bk, Wv, bv, Wo, bo)

    nc = _build()
    in_maps = make_in_maps(Q, K, V, Wq, bq, Wk, bk, Wv, bv, Wo)
    res = run_bass_kernel_spmd(nc, in_maps, list(range(N_CORES))).results
    return gather_output(res, bo)


# revision 14
# speedup vs baseline: 2.7845x; 1.5363x over previous
"""Multi-head attention (B=4, S=2048, D=1024, H=16) on 8 Trainium2 cores.

Sharding: data-parallel over batch (4) x tensor-parallel over heads (2).
Core c handles batch c//2 and heads (c%2)*8 .. +8.  Each core computes a
partial output (its heads' contribution through the O-projection); the host
sums the two partials per batch and adds the output bias.

All device matmuls are arranged so the contraction dim sits on SBUF
partitions; activations are pre-transposed on the host so no on-device
transposes are needed.  Matmul operands are bf16 (PSUM accumulation stays
f32, as does the softmax normalization), which quadruples TensorE
throughput vs fp32's two-pass mode.
"""

import numpy as np
from contextlib import ExitStack

import ml_dtypes
import concourse.bass as bass
import concourse.tile as tile
from concourse import bacc, mybir
from concourse.bass import ts
from concourse.bass_utils import run_bass_kernel_spmd

P = 128
S = 2048          # sequence length
D = 1024          # model dim
DOUT = 512        # per-core projection width (8 heads x 64)
DK = 64           # head dim
B = 4
N_CORES = 8
F32 = mybir.dt.float32
BF16 = mybir.dt.bfloat16
FP = mybir.ActivationFunctionType

NKC = D // P      # 8 contraction chunks over model dim
NM = DOUT // P    # 4 dout chunks (also head pairs)
NQ = S // 512     # 4 query chunks of 512
NK16 = S // P     # 16 key chunks of 128

_cached_nc = None


def _emit(ctx: ExitStack, tc: "tile.TileContext", io: dict):
    nc = tc.nc

    qt_r = io["qt"].ap().rearrange("(c p) s -> p c s", p=P)      # [128, 8, 2048]
    kt_r = io["kt"].ap().rearrange("(c p) s -> p c s", p=P)
    vt_r = io["vt"].ap().rearrange("(c p) s -> p c s", p=P)
    wqt_r = io["wqt"].ap().rearrange("(c p) m -> p c m", p=P)    # [128, 8, 512]
    wkt_r = io["wkt"].ap().rearrange("(c p) m -> p c m", p=P)
    wvt_r = io["wvt"].ap().rearrange("(c p) m -> p c m", p=P)
    wot_r = io["wot"].ap().rearrange("(c p) n -> p c n", p=P)    # [128, 4, 1024]
    bq_r = io["bq"].ap().rearrange("(c p) -> p c", p=P)          # [128, 4]
    bk_r = io["bk"].ap().rearrange("(c p) -> p c", p=P)
    bv_ap = io["bv"].ap()                                        # [512]
    out_r = io["out"].ap().rearrange("(sc p) n -> p sc n", p=P)  # [128, 16, 1024]

    persist = ctx.enter_context(tc.tile_pool(name="persist", bufs=1))
    weights = ctx.enter_context(tc.tile_pool(name="weights", bufs=2))
    streams = ctx.enter_context(tc.tile_pool(name="streams", bufs=3))
    etp = ctx.enter_context(tc.tile_pool(name="etp", bufs=6))
    recipp = ctx.enter_context(tc.tile_pool(name="recipp", bufs=2))
    outp = ctx.enter_context(tc.tile_pool(name="outp", bufs=3))
    dramp = ctx.enter_context(tc.tile_pool(name="dramp", bufs=2, space="DRAM"))

    ps_st = ctx.enter_context(tc.tile_pool(name="ps_st", bufs=4, space="PSUM"))
    ps_proj = ps_st  # proj phases reuse the ST banks (disjoint in time)
    ps_av = ctx.enter_context(tc.tile_pool(name="ps_av", bufs=2, space="PSUM"))
    ps_sums = ctx.enter_context(tc.tile_pool(name="ps_sums", bufs=2, space="PSUM"))

    # ---- constants / biases -------------------------------------------------
    bq_sb = persist.tile([P, NM], F32, tag="bq")
    nc.sync.dma_start(out=bq_sb, in_=bq_r)
    bk_sb = persist.tile([P, NM], F32, tag="bk")
    nc.sync.dma_start(out=bk_sb, in_=bk_r)
    # bv replicated across partitions (DMA partition-broadcast, stride 0)
    bv_rep = persist.tile([P, DOUT], F32, tag="bvrep")
    bv_bcast = bass.AP(
        tensor=bv_ap.tensor, offset=bv_ap.offset, ap=[[0, P]] + list(bv_ap.ap)
    )
    nc.gpsimd.dma_start(out=bv_rep, in_=bv_bcast)
    ones_sb = persist.tile([P, 64], BF16, tag="ones")
    nc.vector.memset(ones_sb, 1.0)

    # ---- persistent activations (bf16) --------------------------------------
    # qT / kT: [dout, s] as 4 chunk-tiles of [128, 2048] (chunk = head pair)
    qT = [persist.tile([P, S], BF16, tag=f"qT{m}", name=f"qT{m}") for m in range(NM)]
    kT = [persist.tile([P, S], BF16, tag=f"kT{m}", name=f"kT{m}") for m in range(NM)]
    # v: [s, dout] as 16 tiles of [128, 512] (tile = 128 seq positions)
    v_sb = [
        persist.tile([P, DOUT], BF16, tag=f"v{i}", name=f"v{i}") for i in range(NK16)
    ]
    # attn_outT: [dout, s] as 4 chunk-tiles (rows 0-63 even head, 64-127 odd)
    aoT = [persist.tile([P, S], BF16, tag=f"aoT{m}", name=f"aoT{m}") for m in range(NM)]

    # ---- projections q^T, k^T -----------------------------------------------
    def project_T(src_r, w_r, bias_sb, dst):
        w_sb = weights.tile([P, NKC, DOUT], BF16, tag="w")
        nc.sync.dma_start(out=w_sb, in_=w_r)
        for si in range(NQ):                      # 512-wide s chunks
            xin = streams.tile([P, NKC, 512], BF16, tag="xin")
            nc.sync.dma_start(out=xin, in_=src_r[:, :, ts(si, 512)])
            for m in range(NM):
                ps = ps_proj.tile([P, 512], F32, tag="st", name="ps")
                for kc in range(NKC):
                    nc.tensor.matmul(
                        ps,
                        lhsT=w_sb[:, kc, ts(m, P)],
                        rhs=xin[:, kc, :],
                        start=(kc == 0),
                        stop=(kc == NKC - 1),
                    )
                nc.vector.tensor_add(
                    out=dst[m][:, ts(si, 512)],
                    in0=ps,
                    in1=bias_sb[:, m : m + 1].to_broadcast([P, 512]),
                )

    project_T(qt_r, wqt_r, bq_sb, qT)
    project_T(kt_r, wkt_r, bk_sb, kT)

    # ---- projection v (natural layout) --------------------------------------
    wv_sb = weights.tile([P, NKC, DOUT], BF16, tag="w")
    nc.sync.dma_start(out=wv_sb, in_=wvt_r)
    for g in range(NQ):
        vin = streams.tile([P, NKC, 512], BF16, tag="xin")
        nc.sync.dma_start(out=vin, in_=vt_r[:, :, ts(g, 512)])
        for j in range(4):
            si16 = g * 4 + j
            ps = ps_proj.tile([P, 512], F32, tag="st", name="psv")
            for kc in range(NKC):
                nc.tensor.matmul(
                    ps,
                    lhsT=vin[:, kc, ts(j, P)],
                    rhs=wv_sb[:, kc, :],
                    start=(kc == 0),
                    stop=(kc == NKC - 1),
                )
            nc.vector.tensor_add(out=v_sb[si16], in0=ps, in1=bv_rep)

    # ---- attention (scores kept transposed: tiles are S^T[k, q]) ------------
    # head pair pc: even head = partitions/cols 0-63, odd head = 64-127
    for pc in range(NM):
        for qi in range(NQ):
            av = ps_av.tile([P, 512], F32, tag="av", name="av")
            sums = ps_sums.tile([P, 512], F32, tag="sums", name="sums")  # rows 0/32
            for ki in range(NK16):
                stA = ps_st.tile([P, 512], F32, tag="st", name="stA")
                stB = ps_st.tile([P, 512], F32, tag="st", name="stB")
                nc.tensor.matmul(
                    stA,
                    lhsT=kT[pc][0:64, ts(ki, P)],
                    rhs=qT[pc][0:64, ts(qi, 512)],
                    start=True,
                    stop=True,
                )
                nc.tensor.matmul(
                    stB,
                    lhsT=kT[pc][64:128, ts(ki, P)],
                    rhs=qT[pc][64:128, ts(qi, 512)],
                    start=True,
                    stop=True,
                )
                etA = etp.tile([P, 512], BF16, tag="et", name="etA")
                etB = etp.tile([P, 512], BF16, tag="et", name="etB")
                nc.scalar.activation(out=etA, in_=stA, func=FP.Exp, scale=0.125)
                nc.scalar.activation(out=etB, in_=stB, func=FP.Exp, scale=0.125)
                first = ki == 0
                last = ki == NK16 - 1
                nc.tensor.matmul(
                    av[0:64],
                    lhsT=v_sb[ki][:, pc * P : pc * P + 64],
                    rhs=etA,
                    start=first,
                    stop=last,
                    skip_group_check=True,
                )
                nc.tensor.matmul(
                    av[64:128],
                    lhsT=v_sb[ki][:, pc * P + 64 : pc * P + 128],
                    rhs=etB,
                    start=first,
                    stop=last,
                    skip_group_check=True,
                )
                nc.tensor.matmul(
                    sums[0:1],
                    lhsT=ones_sb[:, 0:1],
                    rhs=etA,
                    start=first,
                    stop=last,
                    skip_group_check=True,
                )
                nc.tensor.matmul(
                    sums[32:33],
                    lhsT=ones_sb[:, 32:33],
                    rhs=etB,
                    start=first,
                    stop=last,
                    skip_group_check=True,
                )
            # softmax denominators: 1/sums, replicated across partitions via a
            # DRAM bounce (DRAM sources allow stride-0 partition broadcast)
            recip = recipp.tile([P, 512], F32, tag="recip", name="recip")
            nc.vector.reciprocal(out=recip[0:1, :], in_=sums[0:1, :])
            nc.vector.reciprocal(out=recip[32:33, :], in_=sums[32:33, :])
            scr = dramp.tile([2, 512], F32, tag="scr", name="scr")
            nc.sync.dma_start(out=scr[0:1, :], in_=recip[0:1, :])
            nc.sync.dma_start(out=scr[1:2, :], in_=recip[32:33, :])
            rep_sb = recipp.tile([P, 512], F32, tag="rep_sb", name="rep_sb")
            s0 = scr[0:1, :]
            s1 = scr[1:2, :]
            nc.sync.dma_start(
                out=rep_sb[0:64, :],
                in_=bass.AP(
                    tensor=s0.tensor, offset=s0.offset, ap=[[0, 64]] + list(s0.ap[1:])
                ),
            )
            nc.sync.dma_start(
                out=rep_sb[64:128, :],
                in_=bass.AP(
                    tensor=s1.tensor, offset=s1.offset, ap=[[0, 64]] + list(s1.ap[1:])
                ),
            )
            nc.vector.tensor_mul(out=aoT[pc][:, ts(qi, 512)], in0=av, in1=rep_sb)

    # ---- O-projection (partial; host adds the other core's half + bias) -----
    wo_sb = weights.tile([P, NM, D], BF16, tag="w")
    nc.sync.dma_start(out=wo_sb, in_=wot_r)
    for si16 in range(NK16):
        for n2 in range(2):
            ps = ps_proj.tile([P, 512], F32, tag="st", name="psv")
            for c in range(NM):
                nc.tensor.matmul(
                    ps,
                    lhsT=aoT[c][:, ts(si16, P)],
                    rhs=wo_sb[:, c, ts(n2, 512)],
                    start=(c == 0),
                    stop=(c == NM - 1),
                )
            osb = outp.tile([P, 512], F32, tag="osb")
            nc.vector.tensor_copy(out=osb, in_=ps)
            nc.sync.dma_start(out=out_r[:, si16, ts(n2, 512)], in_=osb)


def _build():
    global _cached_nc
    if _cached_nc is not None:
        return _cached_nc
    nc = bacc.Bacc("TRN2", target_bir_lowering=False, debug=False)
    io = {
        "qt": nc.dram_tensor("qt", [D, S], BF16, kind="ExternalInput"),
        "kt": nc.dram_tensor("kt", [D, S], BF16, kind="ExternalInput"),
        "vt": nc.dram_tensor("vt", [D, S], BF16, kind="ExternalInput"),
        "wqt": nc.dram_tensor("wqt", [D, DOUT], BF16, kind="ExternalInput"),
        "wkt": nc.dram_tensor("wkt", [D, DOUT], BF16, kind="ExternalInput"),
        "wvt": nc.dram_tensor("wvt", [D, DOUT], BF16, kind="ExternalInput"),
        "wot": nc.dram_tensor("wot", [DOUT, D], BF16, kind="ExternalInput"),
        "bq": nc.dram_tensor("bq", [DOUT], F32, kind="ExternalInput"),
        "bk": nc.dram_tensor("bk", [DOUT], F32, kind="ExternalInput"),
        "bv": nc.dram_tensor("bv", [DOUT], F32, kind="ExternalInput"),
        "out": nc.dram_tensor("out", [S, D], F32, kind="ExternalOutput"),
    }
    with tile.TileContext(nc) as tc:
        with ExitStack() as ctx:
            _emit(ctx, tc, io)
    nc.compile()
    _cached_nc = nc
    return nc


def make_in_maps(Q, K, V, Wq, bq, Wk, bk, Wv, bv, Wo):
    bf = lambda a: np.ascontiguousarray(np.asarray(a, np.float32)).astype(
        ml_dtypes.bfloat16
    )
    f = lambda a: np.ascontiguousarray(a, dtype=np.float32)
    in_maps = []
    for c in range(N_CORES):
        b = c // 2
        lo = (c % 2) * DOUT
        sl = slice(lo, lo + DOUT)
        in_maps.append(
            {
                "qt": bf(np.asarray(Q, np.float32)[b].T),
                "kt": bf(np.asarray(K, np.float32)[b].T),
                "vt": bf(np.asarray(V, np.float32)[b].T),
                "wqt": bf(np.asarray(Wq, np.float32)[sl, :].T),
                "wkt": bf(np.asarray(Wk, np.float32)[sl, :].T),
                "wvt": bf(np.asarray(Wv, np.float32)[sl, :].T),
                "wot": bf(np.asarray(Wo, np.float32)[:, sl].T),
                "bq": f(bq[sl]),
                "bk": f(bk[sl]),
                "bv": f(bv[sl]),
            }
        )
    return in_maps


def gather_output(results, bo):
    out = np.empty((B, S, D), dtype=np.float32)
    bo = np.asarray(bo, dtype=np.float32)
    for b in range(B):
        out[b] = results[2 * b]["out"] + results[2 * b + 1]["out"] + bo
    return out


def _numpy_fallback(Q, K, V, mask, Wq, bq, Wk, bk, Wv, bv, Wo, bo):
    """Exact reference math in numpy (only used if mask isn't all-ones)."""
    H, dk = 16, 64
    out = np.empty((B, S, D), dtype=np.float32)
    for b in range(B):
        q = (Q[b] @ Wq.T + bq).reshape(S, H, dk).transpose(1, 0, 2)
        k = (K[b] @ Wk.T + bk).reshape(S, H, dk).transpose(1, 0, 2)
        v = (V[b] @ Wv.T + bv).reshape(S, H, dk).transpose(1, 0, 2)
        o = np.empty((H, S, dk), dtype=np.float32)
        for h in range(H):
            s = (q[h] @ k[h].T) / np.sqrt(np.float32(dk))
            s = np.where(mask[b] == 0, np.float32(-1.0e9), s)
            s = s - s.max(axis=-1, keepdims=True)
            e = np.exp(s)
            a = e / e.sum(axis=-1, keepdims=True)
            o[h] = a @ v[h]
        out[b] = o.transpose(1, 0, 2).reshape(S, H * dk) @ Wo.T + bo
    return out


def kernel(Q, K, V, mask, Wq, bq, Wk, bk, Wv, bv, Wo, bo):
    Q = np.asarray(Q, dtype=np.float32)
    K = np.asarray(K, dtype=np.float32)
    V = np.asarray(V, dtype=np.float32)
    Wq = np.asarray(Wq, dtype=np.float32)
    Wk = np.asarray(Wk, dtype=np.float32)
    Wv = np.asarray(Wv, dtype=np.float32)
    Wo = np.asarray(Wo, dtype=np.float32)
    bq = np.asarray(bq, dtype=np.float32)
    bk = np.asarray(bk, dtype=np.float32)
    bv = np.asarray(bv, dtype=np.float32)
    bo = np.asarray(bo, dtype=np.float32)
    mask_np = np.asarray(mask)

    if not np.all(mask_np != 0):
        return _numpy_fallback(Q, K, V, mask_np, Wq, bq, Wk, bk, Wv, bv, Wo, bo)

    nc = _build()
    in_maps = make_in_maps(Q, K, V, Wq, bq, Wk, bk, Wv, bv, Wo)
    res = run_bass_kernel_spmd(nc, in_maps, list(range(N_CORES))).results
    return gather_output(res, bo)


# revision 15
# speedup vs baseline: 4.2445x; 1.5243x over previous
"""Multi-head attention (B=4, S=2048, D=1024, H=16) on 8 Trainium2 cores.

Sharding: data-parallel over batch (4) x tensor-parallel over heads (2).
Core c handles batch c//2 and heads (c%2)*8 .. +8.  Each core computes a
partial output (its heads' contribution through the O-projection); the host
sums the two partials per batch and adds the output bias.

All device matmuls are arranged so the contraction dim sits on SBUF
partitions; activations are pre-transposed on the host so no on-device
transposes are needed.  Matmul operands are bf16 (PSUM accumulation stays
f32, as does the softmax normalization), which quadruples TensorE
throughput vs fp32's two-pass mode.
"""

import numpy as np
from contextlib import ExitStack

import ml_dtypes
import concourse.bass as bass
import concourse.tile as tile
from concourse import bacc, mybir
from concourse.bass import ts
from concourse.bass_utils import run_bass_kernel_spmd

P = 128
S = 2048          # sequence length
D = 1024          # model dim
DOUT = 512        # per-core projection width (8 heads x 64)
DK = 64           # head dim
B = 4
N_CORES = 8
F32 = mybir.dt.float32
BF16 = mybir.dt.bfloat16
FP = mybir.ActivationFunctionType

NKC = D // P      # 8 contraction chunks over model dim
NM = DOUT // P    # 4 dout chunks (also head pairs)
NQ = S // 512     # 4 query chunks of 512
NK16 = S // P     # 16 key chunks of 128

_cached_nc = None


def _emit(ctx: ExitStack, tc: "tile.TileContext", io: dict):
    nc = tc.nc

    qt_r = io["qt"].ap().rearrange("(c p) s -> p c s", p=P)      # [128, 8, 2048]
    kt_r = io["kt"].ap().rearrange("(c p) s -> p c s", p=P)
    vt_r = io["vt"].ap().rearrange("(c p) s -> p c s", p=P)
    wqt_r = io["wqt"].ap().rearrange("(c p) m -> p c m", p=P)    # [128, 8, 512]
    wkt_r = io["wkt"].ap().rearrange("(c p) m -> p c m", p=P)
    wvt_r = io["wvt"].ap().rearrange("(c p) m -> p c m", p=P)
    wot_r = io["wot"].ap().rearrange("(c p) n -> p c n", p=P)    # [128, 4, 1024]
    bq_r = io["bq"].ap().rearrange("(c p) -> p c", p=P)          # [128, 4]
    bk_r = io["bk"].ap().rearrange("(c p) -> p c", p=P)
    bv_ap = io["bv"].ap()                                        # [512]
    out_r = io["out"].ap().rearrange("(sc p) n -> p sc n", p=P)  # [128, 16, 1024]

    persist = ctx.enter_context(tc.tile_pool(name="persist", bufs=1))
    weights = ctx.enter_context(tc.tile_pool(name="weights", bufs=2))
    streams = ctx.enter_context(tc.tile_pool(name="streams", bufs=3))
    etp = ctx.enter_context(tc.tile_pool(name="etp", bufs=6))
    recipp = ctx.enter_context(tc.tile_pool(name="recipp", bufs=2))
    outp = ctx.enter_context(tc.tile_pool(name="outp", bufs=3))
    dramp = ctx.enter_context(tc.tile_pool(name="dramp", bufs=2, space="DRAM"))

    ps_st = ctx.enter_context(tc.tile_pool(name="ps_st", bufs=3, space="PSUM"))
    ps_proj = ps_st  # proj phases reuse the ST banks (disjoint in time)
    ps_av = ctx.enter_context(tc.tile_pool(name="ps_av", bufs=2, space="PSUM"))

    # ---- constants / biases -------------------------------------------------
    bq_sb = persist.tile([P, NM], F32, tag="bq")
    nc.sync.dma_start(out=bq_sb, in_=bq_r)
    bk_sb = persist.tile([P, NM], F32, tag="bk")
    nc.sync.dma_start(out=bk_sb, in_=bk_r)
    # bv replicated across partitions (DMA partition-broadcast, stride 0)
    bv_rep = persist.tile([P, DOUT], F32, tag="bvrep")
    bv_bcast = bass.AP(
        tensor=bv_ap.tensor, offset=bv_ap.offset, ap=[[0, P]] + list(bv_ap.ap)
    )
    nc.gpsimd.dma_start(out=bv_rep, in_=bv_bcast)
    ones_sb = persist.tile([P, 64], BF16, tag="ones")
    nc.vector.memset(ones_sb, 1.0)

    # ---- persistent activations (bf16) --------------------------------------
    # qT / kT: [dout, s] as 4 chunk-tiles of [128, 2048] (chunk = head pair)
    qT = [persist.tile([P, S], BF16, tag=f"qT{m}", name=f"qT{m}") for m in range(NM)]
    kT = [persist.tile([P, S], BF16, tag=f"kT{m}", name=f"kT{m}") for m in range(NM)]
    # v: [s, head, dk+1] tiles; col 64 of each head block holds ones so the
    # AV matmul's 65th output row accumulates the softmax denominator
    v_sb = [
        persist.tile([P, 8, 65], BF16, tag=f"v{i}", name=f"v{i}") for i in range(NK16)
    ]
    for i in range(NK16):
        nc.vector.memset(v_sb[i][:, :, 64:65], 1.0)
    # attn_outT: [dout, s] as 4 chunk-tiles (rows 0-63 even head, 64-127 odd)
    aoT = [persist.tile([P, S], BF16, tag=f"aoT{m}", name=f"aoT{m}") for m in range(NM)]

    # ---- projections q^T, k^T -----------------------------------------------
    def project_T(src_r, w_r, bias_sb, dst):
        w_sb = weights.tile([P, NKC, DOUT], BF16, tag="w")
        nc.sync.dma_start(out=w_sb, in_=w_r)
        for si in range(NQ):                      # 512-wide s chunks
            xin = streams.tile([P, NKC, 512], BF16, tag="xin")
            nc.sync.dma_start(out=xin, in_=src_r[:, :, ts(si, 512)])
            for m in range(NM):
                ps = ps_proj.tile([P, 512], F32, tag="st", name="ps")
                for kc in range(NKC):
                    nc.tensor.matmul(
                        ps,
                        lhsT=w_sb[:, kc, ts(m, P)],
                        rhs=xin[:, kc, :],
                        start=(kc == 0),
                        stop=(kc == NKC - 1),
                    )
                nc.vector.tensor_add(
                    out=dst[m][:, ts(si, 512)],
                    in0=ps,
                    in1=bias_sb[:, m : m + 1].to_broadcast([P, 512]),
                )

    project_T(qt_r, wqt_r, bq_sb, qT)
    project_T(kt_r, wkt_r, bk_sb, kT)

    # ---- projection v (natural layout) --------------------------------------
    wv_sb = weights.tile([P, NKC, DOUT], BF16, tag="w")
    nc.sync.dma_start(out=wv_sb, in_=wvt_r)
    for g in range(NQ):
        vin = streams.tile([P, NKC, 512], BF16, tag="xin")
        nc.sync.dma_start(out=vin, in_=vt_r[:, :, ts(g, 512)])
        for j in range(4):
            si16 = g * 4 + j
            ps = ps_proj.tile([P, 512], F32, tag="st", name="psv")
            for kc in range(NKC):
                nc.tensor.matmul(
                    ps,
                    lhsT=vin[:, kc, ts(j, P)],
                    rhs=wv_sb[:, kc, :],
                    start=(kc == 0),
                    stop=(kc == NKC - 1),
                )
            nc.vector.tensor_add(
                out=v_sb[si16][:, :, 0:64],
                in0=ps.rearrange("p (h d) -> p h d", h=8),
                in1=bv_rep.rearrange("p (h d) -> p h d", h=8),
            )

    # ---- attention (scores kept transposed: tiles are S^T[k, q]) ------------
    # head pair pc: even head = partitions/cols 0-63, odd head = 64-127
    avsb = ctx.enter_context(tc.tile_pool(name="avsb", bufs=4))
    stagp = ctx.enter_context(tc.tile_pool(name="stagp", bufs=2))
    for pc in range(NM):
        hh = 2 * pc
        for qi in range(NQ):
            av_e = ps_av.tile([P, 512], F32, tag="av", name="av_e")  # rows 0-64
            av_o = ps_av.tile([P, 512], F32, tag="av", name="av_o")
            for ki in range(NK16):
                st = ps_st.tile([P, 1024], F32, tag="st", name="st")
                nc.tensor.matmul(
                    st[:, 0:512],
                    lhsT=kT[pc][0:64, ts(ki, P)],
                    rhs=qT[pc][0:64, ts(qi, 512)],
                    start=True,
                    stop=True,
                )
                nc.tensor.matmul(
                    st[:, 512:1024],
                    lhsT=kT[pc][64:128, ts(ki, P)],
                    rhs=qT[pc][64:128, ts(qi, 512)],
                    start=True,
                    stop=True,
                    skip_group_check=True,
                )
                et = etp.tile([P, 1024], BF16, tag="et", name="et")
                nc.scalar.activation(out=et, in_=st, func=FP.Exp, scale=0.125)
                first = ki == 0
                last = ki == NK16 - 1
                nc.tensor.matmul(
                    av_e[0:65],
                    lhsT=v_sb[ki][:, hh, :],
                    rhs=et[:, 0:512],
                    start=first,
                    stop=last,
                    skip_group_check=True,
                )
                nc.tensor.matmul(
                    av_o[0:65],
                    lhsT=v_sb[ki][:, hh + 1, :],
                    rhs=et[:, 512:1024],
                    start=first,
                    stop=last,
                    skip_group_check=True,
                )
            # free the PSUM banks fast, then normalize from SBUF
            ae = avsb.tile([P, 512], F32, tag="ae", name="ae")
            ao = avsb.tile([P, 512], F32, tag="ae", name="ao")
            nc.vector.tensor_copy(out=ae[0:65], in_=av_e[0:65])
            nc.vector.tensor_copy(out=ao[0:65], in_=av_o[0:65])
            recip = recipp.tile([P, 1024], F32, tag="recip", name="recip")
            nc.vector.reciprocal(out=recip[64:65, 0:512], in_=ae[64:65, :])
            nc.vector.reciprocal(out=recip[64:65, 512:1024], in_=ao[64:65, :])
            scr = dramp.tile([2, 512], F32, tag="scr", name="scr")
            nc.sync.dma_start(out=scr[0:1, :], in_=recip[64:65, 0:512])
            nc.sync.dma_start(out=scr[1:2, :], in_=recip[64:65, 512:1024])
            rep = recipp.tile([P, 1024], F32, tag="rep", name="rep")
            s0 = scr[0:1, :]
            s1 = scr[1:2, :]
            nc.sync.dma_start(
                out=rep[0:64, 0:512],
                in_=bass.AP(
                    tensor=s0.tensor, offset=s0.offset, ap=[[0, 64]] + list(s0.ap[1:])
                ),
            )
            nc.sync.dma_start(
                out=rep[0:64, 512:1024],
                in_=bass.AP(
                    tensor=s1.tensor, offset=s1.offset, ap=[[0, 64]] + list(s1.ap[1:])
                ),
            )
            nc.vector.tensor_mul(
                out=aoT[pc][0:64, ts(qi, 512)], in0=ae[0:64], in1=rep[0:64, 0:512]
            )
            stag = stagp.tile([P, 512], BF16, tag="stag", name="stag")
            nc.vector.tensor_mul(
                out=stag[0:64, :], in0=ao[0:64], in1=rep[0:64, 512:1024]
            )
            nc.sync.dma_start(out=aoT[pc][64:128, ts(qi, 512)], in_=stag[0:64, :])

    # ---- O-projection (partial; host adds the other core's half + bias) -----
    wo_sb = weights.tile([P, NM, D], BF16, tag="w")
    nc.sync.dma_start(out=wo_sb, in_=wot_r)
    for si16 in range(NK16):
        for n2 in range(2):
            ps = ps_proj.tile([P, 512], F32, tag="st", name="psv")
            for c in range(NM):
                nc.tensor.matmul(
                    ps,
                    lhsT=aoT[c][:, ts(si16, P)],
                    rhs=wo_sb[:, c, ts(n2, 512)],
                    start=(c == 0),
                    stop=(c == NM - 1),
                )
            osb = outp.tile([P, 512], F32, tag="osb")
            nc.vector.tensor_copy(out=osb, in_=ps)
            nc.sync.dma_start(out=out_r[:, si16, ts(n2, 512)], in_=osb)


def _build():
    global _cached_nc
    if _cached_nc is not None:
        return _cached_nc
    nc = bacc.Bacc("TRN2", target_bir_lowering=False, debug=False)
    io = {
        "qt": nc.dram_tensor("qt", [D, S], BF16, kind="ExternalInput"),
        "kt": nc.dram_tensor("kt", [D, S], BF16, kind="ExternalInput"),
        "vt": nc.dram_tensor("vt", [D, S], BF16, kind="ExternalInput"),
        "wqt": nc.dram_tensor("wqt", [D, DOUT], BF16, kind="ExternalInput"),
        "wkt": nc.dram_tensor("wkt", [D, DOUT], BF16, kind="ExternalInput"),
        "wvt": nc.dram_tensor("wvt", [D, DOUT], BF16, kind="ExternalInput"),
        "wot": nc.dram_tensor("wot", [DOUT, D], BF16, kind="ExternalInput"),
        "bq": nc.dram_tensor("bq", [DOUT], F32, kind="ExternalInput"),
        "bk": nc.dram_tensor("bk", [DOUT], F32, kind="ExternalInput"),
        "bv": nc.dram_tensor("bv", [DOUT], F32, kind="ExternalInput"),
        "out": nc.dram_tensor("out", [S, D], F32, kind="ExternalOutput"),
    }
    with tile.TileContext(nc) as tc:
        with ExitStack() as ctx:
            _emit(ctx, tc, io)
    nc.compile()
    _cached_nc = nc
    return nc


def make_in_maps(Q, K, V, Wq, bq, Wk, bk, Wv, bv, Wo):
    bf = lambda a: np.ascontiguousarray(np.asarray(a, np.float32)).astype(
        ml_dtypes.bfloat16
    )
    f = lambda a: np.ascontiguousarray(a, dtype=np.float32)
    in_maps = []
    for c in range(N_CORES):
        b = c // 2
        lo = (c % 2) * DOUT
        sl = slice(lo, lo + DOUT)
        in_maps.append(
            {
                "qt": bf(np.asarray(Q, np.float32)[b].T),
                "kt": bf(np.asarray(K, np.float32)[b].T),
                "vt": bf(np.asarray(V, np.float32)[b].T),
                "wqt": bf(np.asarray(Wq, np.float32)[sl, :].T),
                "wkt": bf(np.asarray(Wk, np.float32)[sl, :].T),
                "wvt": bf(np.asarray(Wv, np.float32)[sl, :].T),
                "wot": bf(np.asarray(Wo, np.float32)[:, sl].T),
                "bq": f(bq[sl]),
                "bk": f(bk[sl]),
                "bv": f(bv[sl]),
            }
        )
    return in_maps


def gather_output(results, bo):
    out = np.empty((B, S, D), dtype=np.float32)
    bo = np.asarray(bo, dtype=np.float32)
    for b in range(B):
        out[b] = results[2 * b]["out"] + results[2 * b + 1]["out"] + bo
    return out


def _numpy_fallback(Q, K, V, mask, Wq, bq, Wk, bk, Wv, bv, Wo, bo):
    """Exact reference math in numpy (only used if mask isn't all-ones)."""
    H, dk = 16, 64
    out = np.empty((B, S, D), dtype=np.float32)
    for b in range(B):
        q = (Q[b] @ Wq.T + bq).reshape(S, H, dk).transpose(1, 0, 2)
        k = (K[b] @ Wk.T + bk).reshape(S, H, dk).transpose(1, 0, 2)
        v = (V[b] @ Wv.T + bv).reshape(S, H, dk).transpose(1, 0, 2)
        o = np.empty((H, S, dk), dtype=np.float32)
        for h in range(H):
            s = (q[h] @ k[h].T) / np.sqrt(np.float32(dk))
            s = np.where(mask[b] == 0, np.float32(-1.0e9), s)
            s = s - s.max(axis=-1, keepdims=True)
            e = np.exp(s)
            a = e / e.sum(axis=-1, keepdims=True)
            o[h] = a @ v[h]
        out[b] = o.transpose(1, 0, 2).reshape(S, H * dk) @ Wo.T + bo
    return out


def kernel(Q, K, V, mask, Wq, bq, Wk, bk, Wv, bv, Wo, bo):
    Q = np.asarray(Q, dtype=np.float32)
    K = np.asarray(K, dtype=np.float32)
    V = np.asarray(V, dtype=np.float32)
    Wq = np.asarray(Wq, dtype=np.float32)
    Wk = np.asarray(Wk, dtype=np.float32)
    Wv = np.asarray(Wv, dtype=np.float32)
    Wo = np.asarray(Wo, dtype=np.float32)
    bq = np.asarray(bq, dtype=np.float32)
    bk = np.asarray(bk, dtype=np.float32)
    bv = np.asarray(bv, dtype=np.float32)
    bo = np.asarray(bo, dtype=np.float32)
    mask_np = np.asarray(mask)

    if not np.all(mask_np != 0):
        return _numpy_fallback(Q, K, V, mask_np, Wq, bq, Wk, bk, Wv, bv, Wo, bo)

    nc = _build()
    in_maps = make_in_maps(Q, K, V, Wq, bq, Wk, bk, Wv, bv, Wo)
    res = run_bass_kernel_spmd(nc, in_maps, list(range(N_CORES))).results
    return gather_output(res, bo)
